# revision 26
# baseline (speedup 1.0000x reference)
"""Trainium2 Bass kernel for nn_AutoregressiveDecoder (gnn_message_passing).

reference math (N=512, D=256, H=64):
    x = z @ z.T
    M[i,r] = r < i;  colsum = (M @ adj) * M;  degs = max(colsum,1)^-0.5
    base = z @ W1[:256]          (the W1[-1] one-hot helper row is provably
                                  dead: spconv masks row i to zero before it
                                  can propagate)
    per i:  d_i = M[i] * degs[i]            (>=0, zero for r>=i)
            Y_i   = adj @ (d_i * base)       [N,H]
            s_i   = (d_i * relu(Y_i)) @ W2   [N]     (relu(d*Y)=d*relu(Y), d>=0)
            t_i   = d_i * s_i
            S[i]  = d_i * (adj @ t_i)        [N]
    out = x + 0.5*(S + S.T)

Distribution: the vmapped i axis is sharded over 8 cores in interleaved
chunks of 16 (core k gets chunks k, k+8, k+16, k+24) so the triangular
prefix bound b = 16c+16 load-balances: every core sees bounds
{128,256,384,512}. adj/z/W1/W2 replicated. Each core returns its 64
x-rows (xout) plus its S^T column shard (stout); the host assembles
out = x_rows + 0.5*(S^T + S) with a numpy transpose.

v4 key tricks (supplement is only 0.6% of ||x||, so its whole path can
run in fp8; x itself runs bf16 for ~1.7e-3 total rel err):
 - adj / prefix-mask / V / T in fp8e4 (adj+mask are 0/1: EXACT in fp8);
   Y/colsum/O matmuls use DoubleRow perf mode (2 K-blocks per pass:
   halves PE stream time; dim1 of a 3D AP indexes the K-tile pair).
 - W2 never multiplied on-chip: |W2_h| is folded into W1's columns
   host-side, columns permuted so every minuend/subtrahend pair of the
   reduction tree's first level has (pos,neg) signs -> level 1 of the
   tree is one subtract + one add instruction; s_pre falls out of the
   plain add-tree. Saves all 10 [P,1024] W2-mult DVE ops.
 - degs via DVE reciprocal_approx_fast + scalar Sqrt (1 act table).
 - x rows exported right after z@z.T; host does the S add.
 - tail: g=3 reduction runs per-pb, stout DMA per-pb.
"""
import sys

sys.path.insert(0, "/opt/trn_rl_repo")

import numpy as np
import ml_dtypes

N = 512
D = 256
H = 64
NCORES = 8
NI = 16            # i per chunk
NCHUNKS = N // NI  # 32
CPC = NCHUNKS // NCORES  # 4 chunks per core
P = 128
KT = N // P        # 4 partition/K tiles
DT = D // P        # 2 contraction tiles for z
BF = ml_dtypes.bfloat16
F8 = ml_dtypes.float8_e4m3

# smalls (bf16): MTbf (mask for DVE mults) | zTkb (my z cols for x)
SMB_COLS = KT * H + DT * H

_cache = {}


def _chunks_of_core(k):
    return [k + NCORES * g for g in range(CPC)]


def _iset_of_core(k):
    out = []
    for c in _chunks_of_core(k):
        out.extend(range(NI * c, NI * (c + 1)))
    return np.array(out, dtype=np.int64)


def _w2_fold(W1, W2):
    """Fold |W2| into W1's columns and order columns so the reduction
    tree's level-1 pairs (j, j+32) are (majority-sign, minority-sign)
    for j < m and (majority, majority) otherwise.

    Returns (W1f [D,64] fp32, m, flip): s_pre = sign_maj * tree-sum;
    flip=True when the majority sign is negative (handled by negating
    dT2 on-chip)."""
    w2 = W2.reshape(H)
    pos = np.where(w2 > 0)[0]
    neg = np.where(w2 <= 0)[0]
    if len(pos) >= len(neg):
        maj, mino, flip = pos, neg, False
    else:
        maj, mino, flip = neg, pos, True
    m = len(mino)
    # left half: m maj | (32-m) maj ; right half: m mino | (32-m) maj
    rest = maj[m:]
    left = np.concatenate([maj[:m], rest[: 32 - m]])
    right = np.concatenate([mino, rest[32 - m :]])
    perm = np.concatenate([left, right]).astype(np.int64)
    assert perm.shape == (H,)
    W1f = W1[:D, perm] * np.abs(w2)[perm][None, :]
    return np.ascontiguousarray(W1f.astype(np.float32)), m, flip


def _build(m, flip):
    import concourse.bacc as bacc
    import concourse.mybir as mybir
    from concourse import tile

    fp32 = mybir.dt.float32
    bf16 = mybir.dt.bfloat16
    fp8 = mybir.dt.float8e4
    AT = mybir.AluOpType
    AF = mybir.ActivationFunctionType
    DR = mybir.MatmulPerfMode.DoubleRow

    nc = bacc.Bacc("TRN2", target_bir_lowering=False, debug=False, num_devices=NCORES)

    adj_in = nc.dram_tensor("adj8", [N, N], fp8, kind="ExternalInput")
    mt8_in = nc.dram_tensor("mt8", [P, KT * H], fp8, kind="ExternalInput")
    zbf_in = nc.dram_tensor("zbfT", [D, N], bf16, kind="ExternalInput")
    w1_in = nc.dram_tensor("W1bf", [D, H], bf16, kind="ExternalInput")
    smb_in = nc.dram_tensor("smallsbf", [P, SMB_COLS], bf16, kind="ExternalInput")

    xout = nc.dram_tensor("xout", [H, N], fp32, kind="ExternalOutput")
    stout = nc.dram_tensor("stout", [N, H], bf16, kind="ExternalOutput")

    def tree_level1(veng, src3, dst3):
        # src3: [P, X, 64], dst3: [P, X, 32]; pos/neg paired subtract
        if m > 0:
            veng.tensor_tensor(
                out=dst3[:, :, 0:m],
                in0=src3[:, :, 0:m],
                in1=src3[:, :, 32 : 32 + m],
                op=AT.subtract,
            )
        if m < 32:
            veng.tensor_tensor(
                out=dst3[:, :, m:32],
                in0=src3[:, :, m:32],
                in1=src3[:, :, 32 + m : 64],
                op=AT.add,
            )

    with tile.TileContext(nc) as tc:
        with (
            tc.tile_pool(name="const", bufs=1) as cpool,
            tc.tile_pool(name="work", bufs=2) as wpool,
            tc.tile_pool(name="ps", bufs=2, space="PSUM") as pspool,
            tc.tile_pool(name="psw", bufs=1, space="PSUM") as pswpool,
            tc.tile_pool(name="ps2", bufs=2, space="PSUM") as ps2pool,
        ):
            # ---- input DMAs over the 3 DMA-capable queues; adj + mask
            # (colsum critical path) lead each queue ----
            MT8 = cpool.tile([P, KT, H], fp8, tag="MT8")
            nc.sync.dma_start(
                out=MT8[:, :, :], in_=mt8_in.ap().rearrange("p (kt i) -> p kt i", kt=KT)
            )
            G = cpool.tile([P, KT, N], fp8, tag="G")
            nc.sync.dma_start(out=G[:, 0, :], in_=adj_in[0 * P : 1 * P, :])
            nc.sync.dma_start(out=G[:, 3, :], in_=adj_in[3 * P : 4 * P, :])

            smb = cpool.tile([P, SMB_COLS], bf16, tag="smb")
            zT = cpool.tile([P, DT, N], bf16, tag="zT")
            nc.scalar.dma_start(out=G[:, 1, :], in_=adj_in[1 * P : 2 * P, :])
            nc.scalar.dma_start(out=smb[:, :], in_=smb_in[:, :])
            MTf = smb[:, 0 : KT * H].rearrange("p (kt i) -> p kt i", kt=KT)
            zTkb = smb[:, KT * H :].rearrange("p (kt i) -> p kt i", kt=DT)
            nc.scalar.dma_start(out=zT[:, 0, :], in_=zbf_in[0:P, :])
            nc.gpsimd.dma_start(out=G[:, 2, :], in_=adj_in[2 * P : 3 * P, :])
            nc.gpsimd.dma_start(out=zT[:, 1, :], in_=zbf_in[P:D, :])
            W1c = cpool.tile([P, DT, H], bf16, tag="W1c")
            nc.gpsimd.dma_start(
                out=W1c[:, :, :], in_=w1_in.ap().rearrange("(kt p) h -> p kt h", p=P)
            )

            # ---- T (t columns for my 64 i's) + Sqrt act-table warmup ----
            Tb = cpool.tile([P, KT, H], fp8, tag="Tb")
            nc.vector.memset(Tb[:, :, :], 0.0)
            warm = cpool.tile([P, 1], fp32, tag="warm")
            # dummy Sqrt pulls the act table load into the DMA-wait window
            nc.scalar.activation(out=warm[:, :], in_=Tb[:, 0, 0:1], func=AF.Sqrt)

            # ---- per-block pipeline + V conveyor, all support work on DVE
            # (GpSimd streaming concurrently with DVE collapses both
            # engines' throughput -- measured 2-16x -- so it stays idle).
            #
            # colsum_b + base_b (PE) -> mx/recip (DVE) -> sq (scalar) ->
            # V(g) (DVE).  The prefix mask is the identity on every full
            # kt-block (r < 128g <= i), so those V blocks read sq
            # directly; only chunk g's top block needs the masked d
            # (a tiny [P,16] mult). ----
            mx = cpool.tile([P, KT, H], fp32, tag="mx")
            r2 = cpool.tile([P, KT, H], fp32, tag="r2")
            sq = cpool.tile([P, KT, H], fp32, tag="sq")
            dT = cpool.tile([P, KT, H], fp32, tag="dT")
            dT2 = cpool.tile([P, KT, H], fp32, tag="dT2")
            dTtop = cpool.tile([P, KT, NI], fp32, tag="dTtop")
            bsb = cpool.tile([P, KT, H], bf16, tag="bsb")
            Vs = []
            for b in range(KT):
                ps = pspool.tile([P, H], fp32, tag="ps")
                for q in range(KT // 2):
                    nc.tensor.matmul(
                        ps[:, :],
                        G[:, 2 * q : 2 * q + 2, b * P : (b + 1) * P],
                        MT8[:, 2 * q : 2 * q + 2, :],
                        start=(q == 0),
                        stop=(q == KT // 2 - 1),
                        perf_mode=DR,
                    )
                psb = pspool.tile([P, H], fp32, tag="ps")
                for kt in range(DT):
                    nc.tensor.matmul(
                        psb[:, :],
                        zT[:, kt, b * P : (b + 1) * P],
                        W1c[:, kt, :],
                        start=(kt == 0),
                        stop=(kt == DT - 1),
                    )
                nc.vector.tensor_scalar_max(out=mx[:, b, :], in0=ps[:, :], scalar1=1.0)
                nc.vector.reciprocal_approx_fast(out=r2[:, b, :], in_=mx[:, b, :])
                nc.scalar.activation(out=sq[:, b, :], in_=r2[:, b, :], func=AF.Sqrt)
                nc.scalar.activation(out=bsb[:, b, :], in_=psb[:, :], func=AF.Copy)
                # masked d for chunk b's top block only
                icol0 = b * NI
                nc.vector.tensor_tensor(
                    out=dTtop[:, b, :],
                    in0=sq[:, b, icol0 : icol0 + NI],
                    in1=MTf[:, b, icol0 : icol0 + NI],
                    op=AT.mult,
                )
                kts = b + 1
                V = cpool.tile([P, kts, NI, H], fp8, tag=f"V{b}")
                if b > 0:
                    nc.vector.tensor_tensor(
                        out=V[:, 0:b, :, :],
                        in0=bsb[:, 0:b, :].unsqueeze(2).broadcast_to((P, b, NI, H)),
                        in1=sq[:, 0:b, icol0 : icol0 + NI]
                        .unsqueeze(3)
                        .broadcast_to((P, b, NI, H)),
                        op=AT.mult,
                    )
                nc.vector.tensor_tensor(
                    out=V[:, b, :, :],
                    in0=bsb[:, b, :].unsqueeze(1).broadcast_to((P, NI, H)),
                    in1=dTtop[:, b, :].unsqueeze(2).broadcast_to((P, NI, H)),
                    op=AT.mult,
                )
                Vs.append(V)

            # ---- x rows (bf16); export via the scalar queue ----
            xps = pswpool.tile([H, N], fp32, tag="pswide")
            for kt in range(DT):
                nc.tensor.matmul(
                    xps[:, :],
                    zTkb[:, kt, :],
                    zT[:, kt, :],
                    start=(kt == 0),
                    stop=(kt == DT - 1),
                )
            xsb = cpool.tile([H, N], fp32, tag="xsb")
            nc.scalar.activation(out=xsb[:, :], in_=xps[:, :], func=AF.Copy)
            nc.scalar.dma_start(out=xout[:, :], in_=xsb[:, :])

            # full masked d / +-d^2: needed only at the ST/T mults near the
            # tail, so they ride the DVE queue after the V conveyor
            nc.vector.tensor_tensor(
                out=dT[:, :, :], in0=sq[:, :, :], in1=MTf[:, :, :], op=AT.mult
            )
            nc.vector.scalar_tensor_tensor(
                out=dT2[:, :, :],
                in0=r2[:, :, :],
                scalar=(-1.0 if flip else 1.0),
                in1=MTf[:, :, :],
                op0=AT.mult,
                op1=AT.mult,
            )

            # ---- main loop over my 4 chunks ----
            for g in range(CPC):
                kts = g + 1  # prefix bound 128*(g+1)
                icol0 = g * NI
                V = Vs[g]
                RW = cpool.tile([P, kts, NI, H], bf16, tag=f"RW{g}")
                for pb in range(kts):
                    yps = ps2pool.tile([P, NI * H], fp32, tag="ps2")
                    # q outer / cc inner: both 512-wide halves reuse the
                    # stationary G pair (one LDWEIGHTS per q, not per mm)
                    Vf = V.rearrange("p k i h -> p k (i h)")
                    for q in range(kts // 2):
                        for cc in range(2):
                            nc.tensor.matmul(
                                yps[:, cc * 512 : (cc + 1) * 512],
                                G[:, 2 * q : 2 * q + 2, pb * P : (pb + 1) * P],
                                Vf[:, 2 * q : 2 * q + 2, cc * 512 : (cc + 1) * 512],
                                start=(q == 0),
                                stop=(q == kts // 2 - 1 and kts % 2 == 0),
                                perf_mode=DR,
                            )
                    if kts % 2:
                        for cc in range(2):
                            nc.tensor.matmul(
                                yps[:, cc * 512 : (cc + 1) * 512],
                                G[:, kts - 1, pb * P : (pb + 1) * P],
                                Vf[:, kts - 1, cc * 512 : (cc + 1) * 512],
                                start=(kts == 1),
                                stop=True,
                            )
                    # relu + cast bf16 out of PSUM on ScalarE
                    nc.scalar.activation(
                        out=RW[:, pb, :, :].rearrange("p i h -> p (i h)"),
                        in_=yps[:, :],
                        func=AF.Relu,
                    )
                    if g == CPC - 1:
                        # per-pb tree: pipelines against next pb's matmuls
                        tA = wpool.tile([P, NI, H // 2], bf16, tag="trA1")
                        tB = wpool.tile([P, NI, H // 4], bf16, tag="trB1")
                        tree_level1(nc.vector, RW[:, pb, :, :], tA)
                        src = tA
                        w = H // 4
                        step = 1
                        while w >= 1:
                            dst = tB if step % 2 == 1 else tA
                            nc.vector.tensor_tensor(
                                out=dst[:, :, 0:w],
                                in0=src[:, :, 0:w],
                                in1=src[:, :, w : 2 * w],
                                op=AT.add,
                            )
                            src = dst
                            w //= 2
                            step += 1
                        nc.vector.tensor_tensor(
                            out=Tb[:, pb, icol0 : icol0 + NI],
                            in0=src[:, :, 0:1].rearrange("p i h -> p (i h)"),
                            in1=dT2[:, pb, icol0 : icol0 + NI],
                            op=AT.mult,
                        )
                if g < CPC - 1:
                    # batched signed tree over all pbs of this chunk
                    bufA = cpool.tile([P, kts, NI, H // 2], bf16, tag=f"trA{g}")
                    bufB = cpool.tile([P, kts, NI, H // 4], bf16, tag=f"trB{g}")
                    tree_level1(
                        nc.vector,
                        RW.rearrange("p k i h -> p (k i) h"),
                        bufA.rearrange("p k i h -> p (k i) h"),
                    )
                    src = bufA
                    w = H // 4
                    step = 1
                    while w >= 1:
                        dst = bufB if step % 2 == 1 else bufA
                        s3 = src[:, :, :, 0 : 2 * w].rearrange("p k i h -> p (k i) h")
                        d3 = dst[:, :, :, 0:w].rearrange("p k i h -> p (k i) h")
                        nc.vector.tensor_tensor(
                            out=d3,
                            in0=s3[:, :, 0:w],
                            in1=s3[:, :, w : 2 * w],
                            op=AT.add,
                        )
                        src = dst
                        w //= 2
                        step += 1
                    # t = s_pre * (+/-)d^2
                    nc.vector.tensor_tensor(
                        out=Tb[:, 0:kts, icol0 : icol0 + NI],
                        in0=src[:, :, :, 0:1].rearrange("p k i h -> p k (i h)"),
                        in1=dT2[:, 0:kts, icol0 : icol0 + NI],
                        op=AT.mult,
                    )

            # ---- O = adj @ T (DoubleRow) ; ST = d * O ; stout per pb.
            # T columns of groups 0..2 are zero for kt>2, so their O only
            # contracts kt<=2 and runs before g=3's T exists; only the 16
            # g=3 columns (full contraction) sit in the serial tail. ----
            CA = (CPC - 1) * NI  # 48 cols from groups 0..2
            STf = cpool.tile([P, KT, H], bf16, tag="STf")
            for pb in range(KT):
                opsAt = pspool.tile([P, H], fp32, tag="ps")
                opsA = opsAt[:, 0:CA]
                nc.tensor.matmul(
                    opsA[:, :],
                    G[:, 0:2, pb * P : (pb + 1) * P],
                    Tb[:, 0:2, 0:CA],
                    start=True,
                    stop=False,
                    perf_mode=DR,
                )
                nc.tensor.matmul(
                    opsA[:, :],
                    G[:, 2, pb * P : (pb + 1) * P],
                    Tb[:, 2, 0:CA],
                    start=False,
                    stop=True,
                )
                nc.vector.tensor_tensor(
                    out=STf[:, pb, 0:CA],
                    in0=opsA[:, :],
                    in1=dT[:, pb, 0:CA],
                    op=AT.mult,
                )
            for pb in range(KT):
                opsBt = pspool.tile([P, H], fp32, tag="ps")
                opsB = opsBt[:, 0 : H - CA]
                for q in range(KT // 2):
                    nc.tensor.matmul(
                        opsB[:, :],
                        G[:, 2 * q : 2 * q + 2, pb * P : (pb + 1) * P],
                        Tb[:, 2 * q : 2 * q + 2, CA:H],
                        start=(q == 0),
                        stop=(q == KT // 2 - 1),
                        perf_mode=DR,
                    )
                nc.vector.tensor_tensor(
                    out=STf[:, pb, CA:H],
                    in0=opsB[:, :],
                    in1=dT[:, pb, CA:H],
                    op=AT.mult,
                )
                nc.sync.dma_start(
                    out=stout[pb * P : (pb + 1) * P, :], in_=STf[:, pb, :]
                )

    nc.compile()
    return nc


def _get_nc(m, flip):
    key = ("nc", m, flip)
    if key not in _cache:
        _cache[key] = _build(m, flip)
    return _cache[key]


def _prepare_in_maps(z, adj, W1, W2):
    z = np.asarray(z, dtype=np.float32)
    adj = np.asarray(adj, dtype=np.float32)
    W1 = np.asarray(W1, dtype=np.float32)
    W2 = np.asarray(W2, dtype=np.float32)

    adj8 = adj.astype(F8)  # 0/1 values: exact in fp8
    zbfT = np.ascontiguousarray(z.T).astype(BF)
    W1f, m, flip = _w2_fold(W1, W2)
    W1bf = W1f.astype(BF)

    idx = np.arange(N)
    in_maps = []
    for k in range(NCORES):
        iset = _iset_of_core(k)
        MT = (idx[:, None] < iset[None, :]).astype(np.float32)  # [N, 64] r < i
        MT_fold = MT.reshape(KT, P, H).transpose(1, 0, 2).reshape(P, KT * H)
        ztk = (
            zbfT.astype(np.float32)[:, iset]
            .reshape(DT, P, H)
            .transpose(1, 0, 2)
            .reshape(P, DT * H)
        )
        smallsbf = np.concatenate([MT_fold, ztk], axis=1).astype(BF)
        in_maps.append(
            {
                "adj8": adj8,
                "mt8": MT_fold.astype(F8),
                "zbfT": zbfT,
                "W1bf": W1bf,
                "smallsbf": smallsbf,
            }
        )
    return in_maps, m, flip


def kernel(z, adj, W1, W2):
    from concourse import bass_utils

    in_maps, m, flip = _prepare_in_maps(z, adj, W1, W2)
    nc = _get_nc(m, flip)
    res = bass_utils.run_bass_kernel_spmd(
        nc, in_maps, core_ids=list(range(NCORES)), trace=False
    )
    out = np.empty((N, N), dtype=np.float32)
    stf = np.empty((N, N), dtype=np.float32)
    for k in range(NCORES):
        iset = _iset_of_core(k)
        out[iset, :] = res.results[k]["xout"]
        stf[:, iset] = res.results[k]["stout"].astype(np.float32)
    # stf[c, i] = S[i, c]  ->  out += 0.5*(S^T + S)
    out += 0.5 * (stf + stf.T)
    return out
